# revision 15
# baseline (speedup 1.0000x reference)
"""NVFP4-style activation quantizer (nn_ActQuantizer) on 8 TRN2 NeuronCores.

Self-contained: hardcodes shapes/sharding for x of shape (2, 2048, 4096) f32.
Data-parallel: the flat 16.8M-element tensor is split into 8 contiguous
2,097,152-element shards (each [128 partitions x 16384]), one per core.
Groups of 16 contiguous elements stay within a partition row.

Algorithm (bit-exact-path validated vs the fp16/e4m3 reference semantics):
  amax  = group abs-max                        (DVE tensor_reduce, abs=True)
  scale = round_to_e4m3(amax)                  ((bits+0x7FFFF) >>20 <<20)
  r6    = ~6/scale                             (reciprocal_approx_fast on
                                                scale/6; ~51 ulp, only shifts
                                                quantization bin edges)
  f     = fp16(x * r6)                         (DVE scalar_tensor_tensor)
  q5    = fp16(f*(1+2^-11) + 768)              (ACT; magic-add rounds to 0.5)
  ql    = clamp(q5-768, -1, 1)                 (DVE 4x: (sub,min) + (max))
  M     = ((bits(|f|) max 0x3C00) + 0x100) & 0xFE00   (ACT Abs + DVE 4x x2)
        = max(round-to-1-mantissa-bit(|f|), 1)
  q     = ql * M                               (DVE fp16 2x)
  y     = fp16(q * scale/6)                    (DVE TT w/ group broadcast)
Output is fp16 (harness tolerance 2e-2; total L2 ~1.3e-3), halving the
output DMA traffic.
"""
import sys

sys.path.insert(0, "/opt/trn_rl_repo")

import numpy as np

import concourse.bass as bass
import concourse.bacc as bacc
import concourse.mybir as mybir
from concourse import tile
from concourse.bass_utils import run_bass_kernel_spmd

AF = mybir.ActivationFunctionType
ALU = mybir.AluOpType

N_CORES = 8
FULL_SHAPE = (2, 2048, 4096)
TOTAL = 2 * 2048 * 4096            # 16,777,216
PER_CORE = TOTAL // N_CORES        # 2,097,152
P = 128
FD = PER_CORE // P                 # 16384 free elems per partition
FT = 4096
NT = FD // FT                      # 8 tiles
GT = FT // 16                      # 128 groups per tile row

S0 = float(np.float32(1.0) + np.float32(2.0 ** -11))

_cached_nc = None


def build_nc() -> bass.Bass:
    nc = bacc.Bacc("TRN2", target_bir_lowering=False, debug=False)
    x = nc.dram_tensor("x", [P, FD], mybir.dt.float32, kind="ExternalInput")
    out = nc.dram_tensor("out", [P, FD], mybir.dt.float16, kind="ExternalOutput")

    with tile.TileContext(nc) as tc:
        with tc.tile_pool(name="xin", bufs=2) as xin_pool, \
             tc.tile_pool(name="yout", bufs=2) as yout_pool, \
             tc.tile_pool(name="work", bufs=3) as work, \
             tc.tile_pool(name="ofp", bufs=3) as ofp, \
             tc.tile_pool(name="small", bufs=3) as small:
            prev = None
            stash = {}

            def phase_a(t):
                """DMA-in + scale pipeline for tile t (DVE smalls + ACT)."""
                sl = slice(t * FT, (t + 1) * FT)
                xt = xin_pool.tile([P, FT], mybir.dt.float32, tag="x")
                nc.sync.dma_start(out=xt[:], in_=x[:, sl])

                # group amax (DVE, 1x)
                am = small.tile([P, GT], mybir.dt.float32, tag="am")
                nc.vector.tensor_reduce(
                    am[:], xt[:].rearrange("p (g s) -> p g s", s=16),
                    axis=mybir.AxisListType.X, op=ALU.max,
                    apply_absolute_value=True,
                )
                # scale = round_to_e4m3(amax) via HW fp8-e4m3 roundtrip
                # (ACT; TRN e4m3 grid == reference grid in our value range)
                o8 = small.tile([P, GT], mybir.dt.float8e4, tag="o8")
                nc.scalar.activation(o8[:], am[:], AF.Copy)
                # o32 = scale/6 (ACT small); r6 = ~1/o32 = 6/scale (DVE fast)
                o32 = small.tile([P, GT], mybir.dt.float32, tag="o32")
                nc.scalar.activation(o32[:], o8[:], AF.Copy, scale=1.0 / 6.0)
                o16 = small.tile([P, GT], mybir.dt.float16, tag="o16")
                nc.scalar.activation(o16[:], o8[:], AF.Copy, scale=1.0 / 6.0)
                # ofull = o16 broadcast-expanded to a packed [P, FT] fp16
                # tile (ACT 1x; lets the final multiply run 2x on DVE)
                ofull = ofp.tile([P, FT], mybir.dt.float16, tag="of")
                nc.scalar.activation(
                    ofull[:].rearrange("p (g s) -> p g s", s=16),
                    o16[:].unsqueeze(2).broadcast_to((P, GT, 16)), AF.Copy)
                r6 = small.tile([P, GT], mybir.dt.float32, tag="r6")
                nc.vector.reciprocal_approx_fast(out=r6[:], in_=o32[:])
                stash[t] = (xt, r6, ofull, sl)

            def phase_b(t):
                nonlocal prev
                xt, r6, ofull, sl = stash.pop(t)
                # f = fp16(x * r6bcast) (DVE, 1x)
                ft = work.tile([P, FT], mybir.dt.float16, tag="f")
                nc.vector.tensor_tensor(
                    ft[:].rearrange("p (g s) -> p g s", s=16),
                    xt[:].rearrange("p (g s) -> p g s", s=16),
                    r6[:].unsqueeze(2).broadcast_to((P, GT, 16)),
                    ALU.mult,
                )
                # mabs = |f| and q5 = fp16(f*s0 + 768), both on ACT
                m = work.tile([P, FT], mybir.dt.float16, tag="m")
                nc.scalar.activation(m[:], ft[:], AF.Abs)
                q = work.tile([P, FT], mybir.dt.float16, tag="q")
                nc.scalar.activation(q[:], ft[:], AF.Copy, bias=768.0, scale=S0)
                # deferred y of the PREVIOUS tile (packed fp16 2x), then the
                # NEXT tile's scale pipeline — both fill DVE's wait on ACT
                if prev is not None:
                    pq, pof, psl = prev
                    yt = yout_pool.tile([P, FT], mybir.dt.float16, tag="y")
                    nc.vector.tensor_tensor(yt[:], pq[:], pof[:], ALU.mult)
                    nc.sync.dma_start(out=out[:, psl], in_=yt[:])
                if t + 1 < NT:
                    phase_a(t + 1)
                # M = ((bits(|f|) max 0x3C00) + 0x100) & 0xFE00 (DVE 4x x2)
                nc.vector.tensor_scalar(
                    m[:].bitcast(mybir.dt.int16), m[:].bitcast(mybir.dt.int16),
                    0x3C00, 0x100, ALU.max, ALU.add,
                )
                nc.vector.tensor_scalar(
                    m[:].bitcast(mybir.dt.int16), m[:].bitcast(mybir.dt.int16),
                    -0x200, None, ALU.bitwise_and,
                )
                # ql = clamp(q5 - 768, -1, 1) (DVE 4x x2)
                nc.vector.tensor_scalar(q[:], q[:], 768.0, 1.0,
                                        ALU.subtract, ALU.min)
                nc.vector.tensor_scalar(q[:], q[:], -1.0, None, ALU.max)
                # q = ql * M (DVE fp16 2x)
                nc.vector.tensor_tensor(q[:], q[:], m[:], ALU.mult)
                prev = (q, ofull, sl)

            phase_a(0)
            for t in range(NT):
                phase_b(t)
            pq, pof, psl = prev
            yt = yout_pool.tile([P, FT], mybir.dt.float16, tag="y")
            nc.vector.tensor_tensor(yt[:], pq[:], pof[:], ALU.mult)
            nc.sync.dma_start(out=out[:, psl], in_=yt[:])
    nc.compile()
    return nc


def _get_nc() -> bass.Bass:
    global _cached_nc
    if _cached_nc is None:
        _cached_nc = build_nc()
    return _cached_nc


def run(x: np.ndarray, trace: bool = False, **kw):
    """Shard, run SPMD on 8 cores, gather. Returns (out_full, BassKernelResults)."""
    x_flat = np.ascontiguousarray(np.asarray(x, dtype=np.float32)).reshape(-1)
    in_maps = [
        {"x": x_flat[i * PER_CORE:(i + 1) * PER_CORE].reshape(P, FD)}
        for i in range(N_CORES)
    ]
    nc = _get_nc()
    res = run_bass_kernel_spmd(nc, in_maps, core_ids=list(range(N_CORES)),
                               trace=trace, **kw)
    out = np.empty(TOTAL, dtype=np.float32)
    for i in range(N_CORES):
        out[i * PER_CORE:(i + 1) * PER_CORE] = (
            res.results[i]["out"].astype(np.float32).reshape(-1))
    return out.reshape(FULL_SHAPE), res


def kernel(x: np.ndarray) -> np.ndarray:
    out, _ = run(x, trace=False)
    return out


# revision 16
# speedup vs baseline: 1.0118x; 1.0118x over previous
"""NVFP4-style activation quantizer (nn_ActQuantizer) on 8 TRN2 NeuronCores.

Self-contained: hardcodes shapes/sharding for x of shape (2, 2048, 4096) f32.
Data-parallel: the flat 16.8M-element tensor is split into 8 contiguous
2,097,152-element shards (each [128 partitions x 16384]), one per core.
Groups of 16 contiguous elements stay within a partition row.

Algorithm (bit-exact-path validated vs the fp16/e4m3 reference semantics):
  amax  = group abs-max                        (DVE tensor_reduce, abs=True)
  scale = round_to_e4m3(amax)                  ((bits+0x7FFFF) >>20 <<20)
  r6    = ~6/scale                             (reciprocal_approx_fast on
                                                scale/6; ~51 ulp, only shifts
                                                quantization bin edges)
  f     = fp16(x * r6)                         (DVE scalar_tensor_tensor)
  q5    = fp16(f*(1+2^-11) + 768)              (ACT; magic-add rounds to 0.5)
  ql    = clamp(q5-768, -1, 1)                 (DVE 4x: (sub,min) + (max))
  M     = ((bits(|f|) max 0x3C00) + 0x100) & 0xFE00   (ACT Abs + DVE 4x x2)
        = max(round-to-1-mantissa-bit(|f|), 1)
  q     = ql * M                               (DVE fp16 2x)
  y     = fp16(q * scale/6)                    (DVE TT w/ group broadcast)
Output is fp16 (harness tolerance 2e-2; total L2 ~1.3e-3), halving the
output DMA traffic.
"""
import sys

sys.path.insert(0, "/opt/trn_rl_repo")

import numpy as np

import concourse.bass as bass
import concourse.bacc as bacc
import concourse.mybir as mybir
from concourse import tile
from concourse.bass_utils import run_bass_kernel_spmd

AF = mybir.ActivationFunctionType
ALU = mybir.AluOpType

N_CORES = 8
FULL_SHAPE = (2, 2048, 4096)
TOTAL = 2 * 2048 * 4096            # 16,777,216
PER_CORE = TOTAL // N_CORES        # 2,097,152
P = 128
FD = PER_CORE // P                 # 16384 free elems per partition
FT = 2048
NT = FD // FT                      # 8 tiles
GT = FT // 16                      # 128 groups per tile row
CLO = 1344                         # f32-exact f-pass columns (rest: fp16 2x)
GLO = CLO // 16
CHI = FT - CLO

S0 = float(np.float32(1.0) + np.float32(2.0 ** -11))

_cached_nc = None


def build_nc() -> bass.Bass:
    nc = bacc.Bacc("TRN2", target_bir_lowering=False, debug=False)
    x = nc.dram_tensor("x", [P, FD], mybir.dt.float32, kind="ExternalInput")
    out = nc.dram_tensor("out", [P, FD], mybir.dt.float16, kind="ExternalOutput")

    with tile.TileContext(nc) as tc:
        with tc.tile_pool(name="xin", bufs=2) as xin_pool, \
             tc.tile_pool(name="yout", bufs=2) as yout_pool, \
             tc.tile_pool(name="work", bufs=3) as work, \
             tc.tile_pool(name="ofp", bufs=3) as ofp, \
             tc.tile_pool(name="small", bufs=3) as small:
            prev = None
            stash = {}

            def phase_a(t):
                """DMA-in + scale pipeline for tile t (DVE smalls + ACT)."""
                sl = slice(t * FT, (t + 1) * FT)
                xt = xin_pool.tile([P, FT], mybir.dt.float32, tag="x")
                nc.sync.dma_start(out=xt[:], in_=x[:, sl])

                # group amax (DVE, 1x)
                am = small.tile([P, GT], mybir.dt.float32, tag="am")
                nc.vector.tensor_reduce(
                    am[:], xt[:].rearrange("p (g s) -> p g s", s=16),
                    axis=mybir.AxisListType.X, op=ALU.max,
                    apply_absolute_value=True,
                )
                # scale = round_to_e4m3(amax) via HW fp8-e4m3 roundtrip
                # (ACT; TRN e4m3 grid == reference grid in our value range)
                o8 = small.tile([P, GT], mybir.dt.float8e4, tag="o8")
                nc.scalar.activation(o8[:], am[:], AF.Copy)
                # o32 = scale/6 (ACT small); r6 = ~1/o32 = 6/scale (DVE fast)
                o32 = small.tile([P, GT], mybir.dt.float32, tag="o32")
                nc.scalar.activation(o32[:], o8[:], AF.Copy, scale=1.0 / 6.0)
                o16 = small.tile([P, GT], mybir.dt.float16, tag="o16")
                nc.scalar.activation(o16[:], o8[:], AF.Copy, scale=1.0 / 6.0)
                # ofull = o16 broadcast-expanded to a packed [P, FT] fp16
                # tile (ACT 1x; lets the final multiply run 2x on DVE)
                ofull = ofp.tile([P, FT], mybir.dt.float16, tag="of")
                nc.scalar.activation(
                    ofull[:].rearrange("p (g s) -> p g s", s=16),
                    o16[:].unsqueeze(2).broadcast_to((P, GT, 16)), AF.Copy)
                r6 = small.tile([P, GT], mybir.dt.float32, tag="r6")
                nc.vector.reciprocal_approx_fast(out=r6[:], in_=o32[:])
                # upper-column fp16 f-path operands (ready one tile ahead)
                r16 = small.tile([P, GT - GLO], mybir.dt.float16, tag="r16")
                nc.vector.tensor_scalar(r16[:], r6[:, GLO:], 0.0, None, ALU.add)
                x16h = work.tile([P, CHI], mybir.dt.float16, tag="x16h")
                nc.scalar.activation(x16h[:], xt[:, CLO:], AF.Copy)
                rfh = work.tile([P, CHI], mybir.dt.float16, tag="rfh")
                nc.scalar.activation(
                    rfh[:].rearrange("p (g s) -> p g s", s=16),
                    r16[:].unsqueeze(2).broadcast_to((P, GT - GLO, 16)),
                    AF.Copy)
                stash[t] = (xt, r6, ofull, x16h, rfh, sl)

            def phase_b(t):
                nonlocal prev
                xt, r6, ofull, x16h, rfh, sl = stash.pop(t)
                # f: lower cols exact f32 (1x), upper cols fp16 packed (2x)
                ft = work.tile([P, FT], mybir.dt.float16, tag="f")
                nc.vector.tensor_tensor(
                    ft[:, :CLO].rearrange("p (g s) -> p g s", s=16),
                    xt[:, :CLO].rearrange("p (g s) -> p g s", s=16),
                    r6[:, :GLO].unsqueeze(2).broadcast_to((P, GLO, 16)),
                    ALU.mult,
                )
                nc.vector.tensor_tensor(ft[:, CLO:], x16h[:], rfh[:], ALU.mult)
                # mabs = |f| and q5 = fp16(f*s0 + 768), both on ACT
                m = work.tile([P, FT], mybir.dt.float16, tag="m")
                nc.scalar.activation(m[:], ft[:], AF.Abs)
                q = work.tile([P, FT], mybir.dt.float16, tag="q")
                nc.scalar.activation(q[:], ft[:], AF.Copy, bias=768.0, scale=S0)
                # deferred y of the PREVIOUS tile (packed fp16 2x), then the
                # NEXT tile's scale pipeline — both fill DVE's wait on ACT
                if prev is not None:
                    pq, pof, psl = prev
                    yt = yout_pool.tile([P, FT], mybir.dt.float16, tag="y")
                    nc.vector.tensor_tensor(yt[:], pq[:], pof[:], ALU.mult)
                    nc.sync.dma_start(out=out[:, psl], in_=yt[:])
                if t + 1 < NT:
                    phase_a(t + 1)
                # M = ((bits(|f|) max 0x3C00) + 0x100) & 0xFE00 (DVE 4x x2)
                nc.vector.tensor_scalar(
                    m[:].bitcast(mybir.dt.int16), m[:].bitcast(mybir.dt.int16),
                    0x3C00, 0x100, ALU.max, ALU.add,
                )
                nc.vector.tensor_scalar(
                    m[:].bitcast(mybir.dt.int16), m[:].bitcast(mybir.dt.int16),
                    -0x200, None, ALU.bitwise_and,
                )
                # ql = clamp(q5 - 768, -1, 1) (DVE 4x x2)
                nc.vector.tensor_scalar(q[:], q[:], 768.0, 1.0,
                                        ALU.subtract, ALU.min)
                nc.vector.tensor_scalar(q[:], q[:], -1.0, None, ALU.max)
                # q = ql * M (DVE fp16 2x)
                nc.vector.tensor_tensor(q[:], q[:], m[:], ALU.mult)
                prev = (q, ofull, sl)

            phase_a(0)
            for t in range(NT):
                phase_b(t)
            pq, pof, psl = prev
            yt = yout_pool.tile([P, FT], mybir.dt.float16, tag="y")
            nc.vector.tensor_tensor(yt[:], pq[:], pof[:], ALU.mult)
            nc.sync.dma_start(out=out[:, psl], in_=yt[:])
    nc.compile()
    return nc


def _get_nc() -> bass.Bass:
    global _cached_nc
    if _cached_nc is None:
        _cached_nc = build_nc()
    return _cached_nc


def run(x: np.ndarray, trace: bool = False, **kw):
    """Shard, run SPMD on 8 cores, gather. Returns (out_full, BassKernelResults)."""
    x_flat = np.ascontiguousarray(np.asarray(x, dtype=np.float32)).reshape(-1)
    in_maps = [
        {"x": x_flat[i * PER_CORE:(i + 1) * PER_CORE].reshape(P, FD)}
        for i in range(N_CORES)
    ]
    nc = _get_nc()
    res = run_bass_kernel_spmd(nc, in_maps, core_ids=list(range(N_CORES)),
                               trace=trace, **kw)
    out = np.empty(TOTAL, dtype=np.float32)
    for i in range(N_CORES):
        out[i * PER_CORE:(i + 1) * PER_CORE] = (
            res.results[i]["out"].astype(np.float32).reshape(-1))
    return out.reshape(FULL_SHAPE), res


def kernel(x: np.ndarray) -> np.ndarray:
    out, _ = run(x, trace=False)
    return out


# revision 17
# speedup vs baseline: 1.0184x; 1.0066x over previous
"""NVFP4-style activation quantizer (nn_ActQuantizer) on 8 TRN2 NeuronCores.

Self-contained: hardcodes shapes/sharding for x of shape (2, 2048, 4096) f32.
Data-parallel: the flat 16.8M-element tensor is split into 8 contiguous
2,097,152-element shards (each [128 partitions x 16384]), one per core.
Groups of 16 contiguous elements stay within a partition row.

Algorithm (bit-exact-path validated vs the fp16/e4m3 reference semantics):
  amax  = group abs-max                        (DVE tensor_reduce, abs=True)
  scale = round_to_e4m3(amax)                  ((bits+0x7FFFF) >>20 <<20)
  r6    = ~6/scale                             (reciprocal_approx_fast on
                                                scale/6; ~51 ulp, only shifts
                                                quantization bin edges)
  f     = fp16(x * r6)                         (DVE scalar_tensor_tensor)
  q5    = fp16(f*(1+2^-11) + 768)              (ACT; magic-add rounds to 0.5)
  ql    = clamp(q5-768, -1, 1)                 (DVE 4x: (sub,min) + (max))
  M     = ((bits(|f|) max 0x3C00) + 0x100) & 0xFE00   (ACT Abs + DVE 4x x2)
        = max(round-to-1-mantissa-bit(|f|), 1)
  q     = ql * M                               (DVE fp16 2x)
  y     = fp16(q * scale/6)                    (DVE TT w/ group broadcast)
Output is fp16 (harness tolerance 2e-2; total L2 ~1.3e-3), halving the
output DMA traffic.
"""
import sys

sys.path.insert(0, "/opt/trn_rl_repo")

import numpy as np

import concourse.bass as bass
import concourse.bacc as bacc
import concourse.mybir as mybir
from concourse import tile
from concourse.bass_utils import run_bass_kernel_spmd

AF = mybir.ActivationFunctionType
ALU = mybir.AluOpType

N_CORES = 8
FULL_SHAPE = (2, 2048, 4096)
TOTAL = 2 * 2048 * 4096            # 16,777,216
PER_CORE = TOTAL // N_CORES        # 2,097,152
P = 128
FD = PER_CORE // P                 # 16384 free elems per partition
FT = 2048
NT = FD // FT                      # 8 tiles
GT = FT // 16                      # 128 groups per tile row

S0 = float(np.float32(1.0) + np.float32(2.0 ** -11))

_cached_nc = None


def build_nc() -> bass.Bass:
    nc = bacc.Bacc("TRN2", target_bir_lowering=False, debug=False)
    x = nc.dram_tensor("x", [P, FD], mybir.dt.float32, kind="ExternalInput")
    out = nc.dram_tensor("out", [P, FD], mybir.dt.float16, kind="ExternalOutput")

    with tile.TileContext(nc) as tc:
        with tc.tile_pool(name="xin", bufs=2) as xin_pool, \
             tc.tile_pool(name="yout", bufs=2) as yout_pool, \
             tc.tile_pool(name="work", bufs=3) as work, \
             tc.tile_pool(name="ofp", bufs=3) as ofp, \
             tc.tile_pool(name="small", bufs=3) as small:
            prev = None
            stash = {}

            def phase_a(t):
                """DMA-in + scale pipeline for tile t (DVE smalls + ACT)."""
                sl = slice(t * FT, (t + 1) * FT)
                xt = xin_pool.tile([P, FT], mybir.dt.float32, tag="x")
                nc.sync.dma_start(out=xt[:], in_=x[:, sl])

                # group amax (DVE, 1x)
                am = small.tile([P, GT], mybir.dt.float32, tag="am")
                nc.vector.tensor_reduce(
                    am[:], xt[:].rearrange("p (g s) -> p g s", s=16),
                    axis=mybir.AxisListType.X, op=ALU.max,
                    apply_absolute_value=True,
                )
                # scale = round_to_e4m3(amax) via HW fp8-e4m3 roundtrip
                # (ACT; TRN e4m3 grid == reference grid in our value range)
                o8 = small.tile([P, GT], mybir.dt.float8e4, tag="o8")
                nc.scalar.activation(o8[:], am[:], AF.Copy)
                # o32 = scale/6 (ACT small); r6 = ~1/o32 = 6/scale (DVE fast)
                o32 = small.tile([P, GT], mybir.dt.float32, tag="o32")
                nc.scalar.activation(o32[:], o8[:], AF.Copy, scale=1.0 / 6.0)
                o16 = small.tile([P, GT], mybir.dt.float16, tag="o16")
                nc.scalar.activation(o16[:], o8[:], AF.Copy, scale=1.0 / 6.0)
                # ofull = o16 broadcast-expanded to a packed [P, FT] fp16
                # tile (ACT 1x; lets the final multiply run 2x on DVE)
                ofull = ofp.tile([P, FT], mybir.dt.float16, tag="of")
                nc.scalar.activation(
                    ofull[:].rearrange("p (g s) -> p g s", s=16),
                    o16[:].unsqueeze(2).broadcast_to((P, GT, 16)), AF.Copy)
                r6 = small.tile([P, GT], mybir.dt.float32, tag="r6")
                nc.vector.reciprocal_approx_fast(out=r6[:], in_=o32[:])
                stash[t] = (xt, r6, ofull, sl)

            def phase_b(t):
                nonlocal prev
                xt, r6, ofull, sl = stash.pop(t)
                # f = fp16(x * r6bcast) (DVE, 1x)
                ft = work.tile([P, FT], mybir.dt.float16, tag="f")
                nc.vector.tensor_tensor(
                    ft[:].rearrange("p (g s) -> p g s", s=16),
                    xt[:].rearrange("p (g s) -> p g s", s=16),
                    r6[:].unsqueeze(2).broadcast_to((P, GT, 16)),
                    ALU.mult,
                )
                # mabs = |f| and q5 = fp16(f*s0 + 768), both on ACT
                m = work.tile([P, FT], mybir.dt.float16, tag="m")
                nc.scalar.activation(m[:], ft[:], AF.Abs)
                q = work.tile([P, FT], mybir.dt.float16, tag="q")
                nc.scalar.activation(q[:], ft[:], AF.Copy, bias=768.0, scale=S0)
                # deferred y of the PREVIOUS tile (packed fp16 2x), then the
                # NEXT tile's scale pipeline — both fill DVE's wait on ACT
                if prev is not None:
                    pq, pof, psl = prev
                    yt = yout_pool.tile([P, FT], mybir.dt.float16, tag="y")
                    nc.vector.tensor_tensor(yt[:], pq[:], pof[:], ALU.mult)
                    nc.sync.dma_start(out=out[:, psl], in_=yt[:])
                if t + 1 < NT:
                    phase_a(t + 1)
                # M = ((bits(|f|) max 0x3C00) + 0x100) & 0xFE00 (DVE 4x x2)
                nc.vector.tensor_scalar(
                    m[:].bitcast(mybir.dt.int16), m[:].bitcast(mybir.dt.int16),
                    0x3C00, 0x100, ALU.max, ALU.add,
                )
                nc.vector.tensor_scalar(
                    m[:].bitcast(mybir.dt.int16), m[:].bitcast(mybir.dt.int16),
                    -0x200, None, ALU.bitwise_and,
                )
                # ql = clamp(q5 - 768, -1, 1) (DVE 4x x2)
                nc.vector.tensor_scalar(q[:], q[:], 768.0, 1.0,
                                        ALU.subtract, ALU.min)
                nc.vector.tensor_scalar(q[:], q[:], -1.0, None, ALU.max)
                # q = ql * M (DVE fp16 2x)
                nc.vector.tensor_tensor(q[:], q[:], m[:], ALU.mult)
                prev = (q, ofull, sl)

            phase_a(0)
            for t in range(NT):
                phase_b(t)
            pq, pof, psl = prev
            yt = yout_pool.tile([P, FT], mybir.dt.float16, tag="y")
            nc.vector.tensor_tensor(yt[:], pq[:], pof[:], ALU.mult)
            nc.sync.dma_start(out=out[:, psl], in_=yt[:])
    nc.compile()
    return nc


def _get_nc() -> bass.Bass:
    global _cached_nc
    if _cached_nc is None:
        _cached_nc = build_nc()
    return _cached_nc


def run(x: np.ndarray, trace: bool = False, **kw):
    """Shard, run SPMD on 8 cores, gather. Returns (out_full, BassKernelResults)."""
    x_flat = np.ascontiguousarray(np.asarray(x, dtype=np.float32)).reshape(-1)
    in_maps = [
        {"x": x_flat[i * PER_CORE:(i + 1) * PER_CORE].reshape(P, FD)}
        for i in range(N_CORES)
    ]
    nc = _get_nc()
    res = run_bass_kernel_spmd(nc, in_maps, core_ids=list(range(N_CORES)),
                               trace=trace, **kw)
    out = np.empty(TOTAL, dtype=np.float32)
    for i in range(N_CORES):
        out[i * PER_CORE:(i + 1) * PER_CORE] = (
            res.results[i]["out"].astype(np.float32).reshape(-1))
    return out.reshape(FULL_SHAPE), res


def kernel(x: np.ndarray) -> np.ndarray:
    out, _ = run(x, trace=False)
    return out


# revision 20
# speedup vs baseline: 1.0419x; 1.0230x over previous
"""NVFP4-style activation quantizer (nn_ActQuantizer) on 8 TRN2 NeuronCores.

Self-contained: hardcodes shapes/sharding for x of shape (2, 2048, 4096) f32.
Data-parallel: the flat 16.8M-element tensor is split into 8 contiguous
2,097,152-element shards (each [128 partitions x 16384]), one per core.
Groups of 16 contiguous elements stay within a partition row.

Algorithm (bit-exact-path validated vs the fp16/e4m3 reference semantics):
  amax  = group abs-max                        (DVE tensor_reduce, abs=True)
  scale = round_to_e4m3(amax)                  ((bits+0x7FFFF) >>20 <<20)
  r6    = ~6/scale                             (reciprocal_approx_fast on
                                                scale/6; ~51 ulp, only shifts
                                                quantization bin edges)
  f     = fp16(x * r6)                         (DVE scalar_tensor_tensor)
  q5    = fp16(f*(1+2^-11) + 768)              (ACT; magic-add rounds to 0.5)
  ql    = clamp(q5-768, -1, 1)                 (DVE 4x: (sub,min) + (max))
  M     = ((bits(|f|) max 0x3C00) + 0x100) & 0xFE00   (ACT Abs + DVE 4x x2)
        = max(round-to-1-mantissa-bit(|f|), 1)
  q     = ql * M                               (DVE fp16 2x)
  y     = fp16(q * scale/6)                    (DVE TT w/ group broadcast)
Output is fp16 (harness tolerance 2e-2; total L2 ~1.3e-3), halving the
output DMA traffic.
"""
import sys

sys.path.insert(0, "/opt/trn_rl_repo")

import numpy as np

import concourse.bass as bass
import concourse.bacc as bacc
import concourse.mybir as mybir
from concourse import tile
from concourse.bass_utils import run_bass_kernel_spmd

AF = mybir.ActivationFunctionType
ALU = mybir.AluOpType

N_CORES = 8
FULL_SHAPE = (2, 2048, 4096)
TOTAL = 2 * 2048 * 4096            # 16,777,216
PER_CORE = TOTAL // N_CORES        # 2,097,152
P = 128
FD = PER_CORE // P                 # 16384 free elems per partition
FT = 2048
NT = FD // FT                      # 8 tiles
GT = FT // 16                      # 128 groups per tile row

S0 = float(np.float32(1.0) + np.float32(2.0 ** -11))

_cached_nc = None


def build_nc() -> bass.Bass:
    nc = bacc.Bacc("TRN2", target_bir_lowering=False, debug=False)
    x = nc.dram_tensor("x", [P, FD], mybir.dt.float32, kind="ExternalInput")
    out = nc.dram_tensor("out", [P, FD], mybir.dt.float16, kind="ExternalOutput")

    with tile.TileContext(nc) as tc:
        with tc.tile_pool(name="xin", bufs=3) as xin_pool, \
             tc.tile_pool(name="yout", bufs=3) as yout_pool, \
             tc.tile_pool(name="work", bufs=3) as work, \
             tc.tile_pool(name="ofp", bufs=3) as ofp, \
             tc.tile_pool(name="small", bufs=3) as small:
            prev = None
            stash = {}

            def phase_a(t):
                """DMA-in + scale pipeline for tile t (DVE smalls + ACT)."""
                sl = slice(t * FT, (t + 1) * FT)
                xt = xin_pool.tile([P, FT], mybir.dt.float32, tag="x")
                nc.sync.dma_start(out=xt[:], in_=x[:, sl])

                # group amax (DVE, 1x)
                am = small.tile([P, GT], mybir.dt.float32, tag="am")
                nc.vector.tensor_reduce(
                    am[:], xt[:].rearrange("p (g s) -> p g s", s=16),
                    axis=mybir.AxisListType.X, op=ALU.max,
                    apply_absolute_value=True,
                )
                # scale = round_to_e4m3(amax) via HW fp8-e4m3 roundtrip
                # (ACT; TRN e4m3 grid == reference grid in our value range)
                o8 = small.tile([P, GT], mybir.dt.float8e4, tag="o8")
                nc.scalar.activation(o8[:], am[:], AF.Copy)
                # o32 = scale/6 (ACT small); r6 = ~1/o32 = 6/scale (DVE fast)
                o32 = small.tile([P, GT], mybir.dt.float32, tag="o32")
                nc.scalar.activation(o32[:], o8[:], AF.Copy, scale=1.0 / 6.0)
                o16 = small.tile([P, GT], mybir.dt.float16, tag="o16")
                nc.scalar.activation(o16[:], o8[:], AF.Copy, scale=1.0 / 6.0)
                # ofull = o16 broadcast-expanded to a packed [P, FT] fp16
                # tile (ACT 1x; lets the final multiply run 2x on DVE)
                ofull = ofp.tile([P, FT], mybir.dt.float16, tag="of")
                nc.scalar.activation(
                    ofull[:].rearrange("p (g s) -> p g s", s=16),
                    o16[:].unsqueeze(2).broadcast_to((P, GT, 16)), AF.Copy)
                r6 = small.tile([P, GT], mybir.dt.float32, tag="r6")
                nc.vector.reciprocal_approx_fast(out=r6[:], in_=o32[:])
                stash[t] = (xt, r6, ofull, sl)

            def phase_b(t):
                nonlocal prev
                xt, r6, ofull, sl = stash.pop(t)
                # f = fp16(x * r6bcast) (DVE, 1x)
                ft = work.tile([P, FT], mybir.dt.float16, tag="f")
                nc.vector.tensor_tensor(
                    ft[:].rearrange("p (g s) -> p g s", s=16),
                    xt[:].rearrange("p (g s) -> p g s", s=16),
                    r6[:].unsqueeze(2).broadcast_to((P, GT, 16)),
                    ALU.mult,
                )
                # mabs = |f| and q5 = fp16(f*s0 + 768), both on ACT
                m = work.tile([P, FT], mybir.dt.float16, tag="m")
                nc.scalar.activation(m[:], ft[:], AF.Abs)
                q = work.tile([P, FT], mybir.dt.float16, tag="q")
                nc.scalar.activation(q[:], ft[:], AF.Copy, bias=768.0, scale=S0)
                # deferred y of the PREVIOUS tile (packed fp16 2x), then the
                # NEXT tile's scale pipeline — both fill DVE's wait on ACT
                if prev is not None:
                    pq, pof, psl = prev
                    yt = yout_pool.tile([P, FT], mybir.dt.float16, tag="y")
                    nc.vector.tensor_tensor(yt[:], pq[:], pof[:], ALU.mult)
                    nc.sync.dma_start(out=out[:, psl], in_=yt[:])
                if t + 1 < NT:
                    phase_a(t + 1)
                # M = ((bits(|f|) max 0x3C00) + 0x100) & 0xFE00 (DVE 4x x2)
                nc.vector.tensor_scalar(
                    m[:].bitcast(mybir.dt.int16), m[:].bitcast(mybir.dt.int16),
                    0x3C00, 0x100, ALU.max, ALU.add,
                )
                nc.vector.tensor_scalar(
                    m[:].bitcast(mybir.dt.int16), m[:].bitcast(mybir.dt.int16),
                    -0x200, None, ALU.bitwise_and,
                )
                # ql = clamp(q5 - 768, -1, 1) (DVE 4x x2)
                nc.vector.tensor_scalar(q[:], q[:], 768.0, 1.0,
                                        ALU.subtract, ALU.min)
                nc.vector.tensor_scalar(q[:], q[:], -1.0, None, ALU.max)
                # q = ql * M (DVE fp16 2x)
                nc.vector.tensor_tensor(q[:], q[:], m[:], ALU.mult)
                prev = (q, ofull, sl)

            phase_a(0)
            for t in range(NT):
                phase_b(t)
            pq, pof, psl = prev
            yt = yout_pool.tile([P, FT], mybir.dt.float16, tag="y")
            nc.vector.tensor_tensor(yt[:], pq[:], pof[:], ALU.mult)
            nc.sync.dma_start(out=out[:, psl], in_=yt[:])
    nc.compile()
    return nc


def _get_nc() -> bass.Bass:
    global _cached_nc
    if _cached_nc is None:
        _cached_nc = build_nc()
    return _cached_nc


def run(x: np.ndarray, trace: bool = False, **kw):
    """Shard, run SPMD on 8 cores, gather. Returns (out_full, BassKernelResults)."""
    x_flat = np.ascontiguousarray(np.asarray(x, dtype=np.float32)).reshape(-1)
    in_maps = [
        {"x": x_flat[i * PER_CORE:(i + 1) * PER_CORE].reshape(P, FD)}
        for i in range(N_CORES)
    ]
    nc = _get_nc()
    res = run_bass_kernel_spmd(nc, in_maps, core_ids=list(range(N_CORES)),
                               trace=trace, **kw)
    out = np.empty(TOTAL, dtype=np.float32)
    for i in range(N_CORES):
        out[i * PER_CORE:(i + 1) * PER_CORE] = (
            res.results[i]["out"].astype(np.float32).reshape(-1))
    return out.reshape(FULL_SHAPE), res


def kernel(x: np.ndarray) -> np.ndarray:
    out, _ = run(x, trace=False)
    return out


# revision 21
# speedup vs baseline: 1.0506x; 1.0084x over previous
"""NVFP4-style activation quantizer (nn_ActQuantizer) on 8 TRN2 NeuronCores.

Self-contained: hardcodes shapes/sharding for x of shape (2, 2048, 4096) f32.
Data-parallel: the flat 16.8M-element tensor is split into 8 contiguous
2,097,152-element shards (each [128 partitions x 16384]), one per core.
Groups of 16 contiguous elements stay within a partition row.

Algorithm (bit-exact-path validated vs the fp16/e4m3 reference semantics):
  amax  = group abs-max                        (DVE tensor_reduce, abs=True)
  scale = round_to_e4m3(amax)                  ((bits+0x7FFFF) >>20 <<20)
  r6    = ~6/scale                             (reciprocal_approx_fast on
                                                scale/6; ~51 ulp, only shifts
                                                quantization bin edges)
  f     = fp16(x * r6)                         (DVE scalar_tensor_tensor)
  q5    = fp16(f*(1+2^-11) + 768)              (ACT; magic-add rounds to 0.5)
  ql    = clamp(q5-768, -1, 1)                 (DVE 4x: (sub,min) + (max))
  M     = ((bits(|f|) max 0x3C00) + 0x100) & 0xFE00   (ACT Abs + DVE 4x x2)
        = max(round-to-1-mantissa-bit(|f|), 1)
  q     = ql * M                               (DVE fp16 2x)
  y     = fp16(q * scale/6)                    (DVE TT w/ group broadcast)
Output is fp16 (harness tolerance 2e-2; total L2 ~1.3e-3), halving the
output DMA traffic.
"""
import sys

sys.path.insert(0, "/opt/trn_rl_repo")

import numpy as np

import concourse.bass as bass
import concourse.bacc as bacc
import concourse.mybir as mybir
from concourse import tile
from concourse.bass_utils import run_bass_kernel_spmd

AF = mybir.ActivationFunctionType
ALU = mybir.AluOpType

N_CORES = 8
FULL_SHAPE = (2, 2048, 4096)
TOTAL = 2 * 2048 * 4096            # 16,777,216
PER_CORE = TOTAL // N_CORES        # 2,097,152
P = 128
FD = PER_CORE // P                 # 16384 free elems per partition
FT = 2048
NT = FD // FT                      # 8 tiles
GT = FT // 16                      # 128 groups per tile row

S0 = float(np.float32(1.0) + np.float32(2.0 ** -11))

_cached_nc = None


def build_nc() -> bass.Bass:
    nc = bacc.Bacc("TRN2", target_bir_lowering=False, debug=False)
    x = nc.dram_tensor("x", [P, FD], mybir.dt.float32, kind="ExternalInput")
    out = nc.dram_tensor("out", [P, FD], mybir.dt.float16, kind="ExternalOutput")

    with tile.TileContext(nc) as tc:
        with tc.tile_pool(name="xin", bufs=3) as xin_pool, \
             tc.tile_pool(name="yout", bufs=3) as yout_pool, \
             tc.tile_pool(name="work", bufs=4) as work, \
             tc.tile_pool(name="ofp", bufs=4) as ofp, \
             tc.tile_pool(name="small", bufs=4) as small:
            prev = None
            stash = {}

            def phase_a(t):
                """DMA-in + scale pipeline for tile t (DVE smalls + ACT)."""
                sl = slice(t * FT, (t + 1) * FT)
                xt = xin_pool.tile([P, FT], mybir.dt.float32, tag="x")
                nc.sync.dma_start(out=xt[:], in_=x[:, sl])

                # group amax (DVE, 1x)
                am = small.tile([P, GT], mybir.dt.float32, tag="am")
                nc.vector.tensor_reduce(
                    am[:], xt[:].rearrange("p (g s) -> p g s", s=16),
                    axis=mybir.AxisListType.X, op=ALU.max,
                    apply_absolute_value=True,
                )
                # scale = round_to_e4m3(amax) via HW fp8-e4m3 roundtrip
                # (ACT; TRN e4m3 grid == reference grid in our value range)
                o8 = small.tile([P, GT], mybir.dt.float8e4, tag="o8")
                nc.scalar.activation(o8[:], am[:], AF.Copy)
                # o32 = scale/6 (ACT small); r6 = ~1/o32 = 6/scale (DVE fast)
                o32 = small.tile([P, GT], mybir.dt.float32, tag="o32")
                nc.scalar.activation(o32[:], o8[:], AF.Copy, scale=1.0 / 6.0)
                o16 = small.tile([P, GT], mybir.dt.float16, tag="o16")
                nc.scalar.activation(o16[:], o8[:], AF.Copy, scale=1.0 / 6.0)
                # ofull = o16 broadcast-expanded to a packed [P, FT] fp16
                # tile (ACT 1x; lets the final multiply run 2x on DVE)
                ofull = ofp.tile([P, FT], mybir.dt.float16, tag="of")
                nc.scalar.activation(
                    ofull[:].rearrange("p (g s) -> p g s", s=16),
                    o16[:].unsqueeze(2).broadcast_to((P, GT, 16)), AF.Copy)
                r6 = small.tile([P, GT], mybir.dt.float32, tag="r6")
                nc.vector.reciprocal_approx_fast(out=r6[:], in_=o32[:])
                stash[t] = (xt, r6, ofull, sl)

            def phase_b(t):
                nonlocal prev
                xt, r6, ofull, sl = stash.pop(t)
                # f = fp16(x * r6bcast) (DVE, 1x)
                ft = work.tile([P, FT], mybir.dt.float16, tag="f")
                nc.vector.tensor_tensor(
                    ft[:].rearrange("p (g s) -> p g s", s=16),
                    xt[:].rearrange("p (g s) -> p g s", s=16),
                    r6[:].unsqueeze(2).broadcast_to((P, GT, 16)),
                    ALU.mult,
                )
                # mabs = |f| and q5 = fp16(f*s0 + 768), both on ACT
                m = work.tile([P, FT], mybir.dt.float16, tag="m")
                nc.scalar.activation(m[:], ft[:], AF.Abs)
                q = work.tile([P, FT], mybir.dt.float16, tag="q")
                nc.scalar.activation(q[:], ft[:], AF.Copy, bias=768.0, scale=S0)
                # deferred y of the PREVIOUS tile (packed fp16 2x), then the
                # NEXT tile's scale pipeline — both fill DVE's wait on ACT
                if prev is not None:
                    pq, pof, psl = prev
                    yt = yout_pool.tile([P, FT], mybir.dt.float16, tag="y")
                    nc.vector.tensor_tensor(yt[:], pq[:], pof[:], ALU.mult)
                    nc.sync.dma_start(out=out[:, psl], in_=yt[:])
                if t + 1 < NT:
                    phase_a(t + 1)
                # M = ((bits(|f|) max 0x3C00) + 0x100) & 0xFE00 (DVE 4x x2)
                nc.vector.tensor_scalar(
                    m[:].bitcast(mybir.dt.int16), m[:].bitcast(mybir.dt.int16),
                    0x3C00, 0x100, ALU.max, ALU.add,
                )
                nc.vector.tensor_scalar(
                    m[:].bitcast(mybir.dt.int16), m[:].bitcast(mybir.dt.int16),
                    -0x200, None, ALU.bitwise_and,
                )
                # ql = clamp(q5 - 768, -1, 1) (DVE 4x x2)
                nc.vector.tensor_scalar(q[:], q[:], 768.0, 1.0,
                                        ALU.subtract, ALU.min)
                nc.vector.tensor_scalar(q[:], q[:], -1.0, None, ALU.max)
                # q = ql * M (DVE fp16 2x)
                nc.vector.tensor_tensor(q[:], q[:], m[:], ALU.mult)
                prev = (q, ofull, sl)

            phase_a(0)
            for t in range(NT):
                phase_b(t)
            pq, pof, psl = prev
            yt = yout_pool.tile([P, FT], mybir.dt.float16, tag="y")
            nc.vector.tensor_tensor(yt[:], pq[:], pof[:], ALU.mult)
            nc.sync.dma_start(out=out[:, psl], in_=yt[:])
    nc.compile()
    return nc


def _get_nc() -> bass.Bass:
    global _cached_nc
    if _cached_nc is None:
        _cached_nc = build_nc()
    return _cached_nc


def run(x: np.ndarray, trace: bool = False, **kw):
    """Shard, run SPMD on 8 cores, gather. Returns (out_full, BassKernelResults)."""
    x_flat = np.ascontiguousarray(np.asarray(x, dtype=np.float32)).reshape(-1)
    in_maps = [
        {"x": x_flat[i * PER_CORE:(i + 1) * PER_CORE].reshape(P, FD)}
        for i in range(N_CORES)
    ]
    nc = _get_nc()
    res = run_bass_kernel_spmd(nc, in_maps, core_ids=list(range(N_CORES)),
                               trace=trace, **kw)
    out = np.empty(TOTAL, dtype=np.float32)
    for i in range(N_CORES):
        out[i * PER_CORE:(i + 1) * PER_CORE] = (
            res.results[i]["out"].astype(np.float32).reshape(-1))
    return out.reshape(FULL_SHAPE), res


def kernel(x: np.ndarray) -> np.ndarray:
    out, _ = run(x, trace=False)
    return out
